# revision 9
# baseline (speedup 1.0000x reference)
"""Trainium2 Bass kernel for nn_AttentionMechanism (batched attention with
per-sample queries), data-parallel across 8 NeuronCores.

Math (per batch row b):
    q = msgs @ Wq.T + bq                         [H]
    k_t = Wk @ tau_t + bk ; scores_t = q.k_t/32
    alpha = softmax(scores) ; out = sum_t alpha_t (Wv @ tau_t + bv)

Rewrite used (exact up to softmax shift invariance):
    qk   = msgs @ (Wq.T @ Wk) + bq @ Wk          [TAU]   (q.bk const in t -> cancels)
    scores_t = qk . tau_t / 32
    p_t  = exp(scores_t)            (scores are O(1), no max-subtraction needed)
    ctx  = sum_t p_t tau_t / sum_t p_t
    out  = ctx @ Wv.T + bv          (uses sum alpha = 1)

Host precomputes the batch-independent weight products (Wfused = Wq.T @ Wk,
qk_bias = bq @ Wk, WvT = Wv.T) and packs them bf16, so the device streams tau
once from HBM (32 MB/core) plus ~1.5 MB of weights.

Device schedule per 128-row b-tile, per t-chunk of 8 trajectory steps:
  DMA   : chunk [128, 8, 1024] f32->bf16 cast on load (~11.1 us, the bound)
  Vector: 1x broadcast mult prod = chunk * qk_rep; 1x 3D tensor_reduce for the
          last 5 scores
  Scalar: 3x activation-accum for the first 3 scores; 2x exp;
          8x diag build (diag_t = ident * p_t via activation scale)
  PE    : 16x matmul ctx[:, bank] += diag(p_t) @ chunk_t  (p_t scaling folded
          into the matmul weights, so no separate p_t*tau elementwise pass)
"""

import math

import numpy as np
import ml_dtypes

import concourse.bass as bass
import concourse.bacc as bacc
import concourse.tile as tile
from concourse import mybir
from concourse.bass_utils import run_bass_kernel_spmd
from concourse.masks import make_identity

F32 = mybir.dt.float32
BF16 = mybir.dt.bfloat16
NP_BF16 = ml_dtypes.bfloat16

B = 2048
T = 32
TAU = 1024
MSG = 512
HID = 1024
VDIM = 128
N_CORES = 8
B_LOCAL = B // N_CORES

Alu = mybir.AluOpType
Act = mybir.ActivationFunctionType


def build(b_local=B_LOCAL, t_chunk=8, chunk_bufs=4, n_scalar_red=4):
    assert b_local % 128 == 0 and T % t_chunk == 0
    n_btiles = b_local // 128
    n_chunks = T // t_chunk

    nc = bacc.Bacc("TRN2", target_bir_lowering=False, debug=False)

    traj = nc.declare_dram_parameter(
        "imagined_trajectory", [b_local, T * TAU], F32, isOutput=False
    )
    msgsT = nc.declare_dram_parameter("msgsT", [MSG, b_local], BF16, isOutput=False)
    Wfused = nc.declare_dram_parameter("Wfused", [MSG, TAU], BF16, isOutput=False)
    qkbias = nc.declare_dram_parameter("qkbias", [TAU], BF16, isOutput=False)
    WvT = nc.declare_dram_parameter("WvT", [TAU, VDIM], BF16, isOutput=False)
    bv = nc.declare_dram_parameter("bv", [VDIM], F32, isOutput=False)
    out = nc.declare_dram_parameter("out", [b_local, VDIM], F32, isOutput=True)

    MQ = MSG // 128  # 4 m-chunks
    CQ = TAU // 128  # 8 c-chunks

    with tile.TileContext(nc) as tc:
        with (
            tc.tile_pool(name="const", bufs=1) as const,
            tc.tile_pool(name="persist", bufs=1) as persist,
            tc.tile_pool(name="psum_big", bufs=2, space="PSUM") as psum_big,
            tc.tile_pool(name="psum_tr", bufs=2, space="PSUM") as psum_tr,
            tc.tile_pool(name="psum_out", bufs=2, space="PSUM") as psum_out,
        ):
            # weights needed for qk first (critical path to first chunk compute)
            Wfused_b = const.tile([128, MQ, TAU], BF16)
            nc.gpsimd.dma_start(
                out=Wfused_b, in_=Wfused[:, :].rearrange("(j p) c -> p j c", p=128)
            )
            msgsT_b = const.tile([128, MQ, b_local], BF16)
            nc.gpsimd.dma_start(
                out=msgsT_b, in_=msgsT[:, :].rearrange("(j p) b -> p j b", p=128)
            )
            qkb_sb = const.tile([1, TAU], BF16)
            nc.gpsimd.dma_start(out=qkb_sb, in_=qkbias[None, :])
            WvT_b = const.tile([128, CQ, VDIM], BF16)
            nc.gpsimd.dma_start(
                out=WvT_b, in_=WvT[:, :].rearrange("(j p) d -> p j d", p=128)
            )
            bv_sb = const.tile([1, VDIM], F32)
            nc.sync.dma_start(out=bv_sb, in_=bv[None, :])

            ident_f = const.tile([128, 128], F32)
            make_identity(nc, ident_f)
            ident_b = const.tile([128, 128], BF16)
            make_identity(nc, ident_b)
            onespad_b = const.tile([128, 128], BF16)
            nc.vector.memset(onespad_b, 0.0)
            nc.vector.memset(onespad_b[0:1, :], 1.0)
            bvpad_b = const.tile([128, VDIM], BF16)
            nc.vector.memset(bvpad_b, 0.0)
            nc.vector.tensor_copy(out=bvpad_b[0:1, :], in_=bv_sb)
            ones_row = const.tile([1, b_local], BF16)
            nc.vector.memset(ones_row, 1.0)

            # qk[b, c] = msgs @ Wfused + qk_bias, scaled by 1/sqrt(H)
            qk_b = [
                persist.tile([128, TAU], BF16, tag=f"qkb{i}", name=f"qkb{i}")
                for i in range(n_btiles)
            ]
            for bi in range(n_btiles):
                bsl = slice(bi * 128, (bi + 1) * 128)
                pq = psum_big.tile([128, TAU], F32, tag="ctx", name="pq")
                for nh in range(2):
                    nsl = slice(nh * 512, (nh + 1) * 512)
                    for mi in range(MQ):
                        nc.tensor.matmul(
                            pq[:, nsl],
                            lhsT=msgsT_b[:, mi, bsl],
                            rhs=Wfused_b[:, mi, nsl],
                            start=(mi == 0),
                            stop=False,
                        )
                    nc.tensor.matmul(
                        pq[:, nsl],
                        lhsT=ones_row[:, bsl],
                        rhs=qkb_sb[:, nsl],
                        start=False,
                        stop=True,
                    )
                nc.scalar.mul(out=qk_b[bi], in_=pq, mul=1.0 / math.sqrt(HID))

            # ---------- main loop: stream tau ----------
            with (
                tc.tile_pool(name="stream", bufs=chunk_bufs) as stream,
                tc.tile_pool(name="bfp", bufs=2) as bfp,
                tc.tile_pool(name="dpool", bufs=2) as dpool,
                tc.tile_pool(name="spool", bufs=4) as spool,
                tc.tile_pool(name="aux", bufs=2) as aux,
            ):
                dumm = aux.tile([128, TAU], BF16, tag="dumm", name="dumm", bufs=1)
                n_vec_red = t_chunk - n_scalar_red
                for bi in range(n_btiles):
                    bsl = slice(bi * 128, (bi + 1) * 128)
                    ctx_ps = psum_big.tile([128, TAU], F32, tag="ctx", name="ctx_ps")
                    p_all = aux.tile([128, T], F32, tag="p", name="p_all")

                    # issue all chunk DMAs for this b-tile up front so later
                    # gpsimd work (diag builds) can't delay prefetch
                    chunks = []
                    for ci in range(n_chunks):
                        chunk_bf = stream.tile(
                            [128, t_chunk, TAU], BF16, tag="chunk", name="chunk_bf"
                        )
                        c0 = ci * t_chunk * TAU
                        nc.gpsimd.dma_start(
                            out=chunk_bf,
                            in_=traj[bsl, c0 : c0 + t_chunk * TAU].rearrange(
                                "p (t c) -> p t c", t=t_chunk
                            ),
                        )
                        chunks.append(chunk_bf)

                    for ci in range(n_chunks):
                        chunk_bf = chunks[ci]
                        # prod = chunk * qk (broadcast over t) in one DVE pass
                        prod = bfp.tile(
                            [128, t_chunk, TAU], BF16, tag="prod", name="prod"
                        )
                        qk_rep = bass.AP(
                            tensor=qk_b[bi].tensor,
                            offset=qk_b[bi].offset,
                            ap=[qk_b[bi].ap[0], [0, t_chunk], [1, TAU]],
                        )
                        nc.vector.tensor_tensor(
                            out=prod, in0=chunk_bf, in1=qk_rep, op=Alu.mult
                        )
                        # scores: first n_scalar_red slices on scalar (activation
                        # accumulator), the rest in one 3D vector tensor_reduce
                        scores_sc = spool.tile(
                            [128, n_scalar_red], F32, tag="ssc", name="scores_sc"
                        )
                        scores_ve = spool.tile(
                            [128, n_vec_red], F32, tag="sve", name="scores_ve"
                        )
                        for tt in range(n_scalar_red):
                            nc.scalar.activation(
                                out=dumm,
                                in_=prod[:, tt, :],
                                func=Act.Copy,
                                accum_out=scores_sc[:, tt : tt + 1],
                            )
                        nc.vector.tensor_reduce(
                            out=scores_ve,
                            in_=prod[:, n_scalar_red:, :],
                            axis=mybir.AxisListType.X,
                            op=Alu.add,
                        )
                        c0t = ci * t_chunk
                        nc.scalar.activation(
                            out=p_all[:, c0t : c0t + n_scalar_red],
                            in_=scores_sc,
                            func=Act.Exp,
                        )
                        nc.scalar.activation(
                            out=p_all[:, c0t + n_scalar_red : c0t + t_chunk],
                            in_=scores_ve,
                            func=Act.Exp,
                        )
                        # diag blocks on gpsimd: diag_all[:, t, :] = ident * p_t
                        diag_all = dpool.tile(
                            [128, t_chunk, 128], BF16, tag="diag", name="diag_all"
                        )
                        ident_rep = bass.AP(
                            tensor=ident_b.tensor,
                            offset=ident_b.offset,
                            ap=[ident_b.ap[0], [0, t_chunk], [1, 128]],
                        )
                        p_sl = p_all[:, c0t : c0t + t_chunk]
                        p_rep = bass.AP(
                            tensor=p_sl.tensor,
                            offset=p_sl.offset,
                            ap=[p_sl.ap[0], p_sl.ap[1], [0, 128]],
                        )
                        nc.gpsimd.tensor_tensor(
                            out=diag_all, in0=ident_rep, in1=p_rep, op=Alu.mult
                        )
                        for tt in range(t_chunk):
                            first = ci == 0 and tt == 0
                            last = ci == n_chunks - 1 and tt == t_chunk - 1
                            for nh in range(2):
                                nc.tensor.matmul(
                                    ctx_ps[:, nh * 512 : (nh + 1) * 512],
                                    lhsT=diag_all[:, tt, :],
                                    rhs=chunk_bf[:, tt, nh * 512 : (nh + 1) * 512],
                                    start=first,
                                    stop=last,
                                )

                    # normalize, project: out = (ctx / sum p) @ Wv.T + bv
                    s_sum = aux.tile([128, 1], F32, tag="ssum", name="s_sum")
                    nc.vector.tensor_reduce(
                        out=s_sum, in_=p_all, axis=mybir.AxisListType.X, op=Alu.add
                    )
                    rinv = aux.tile([128, 1], F32, tag="rinv", name="rinv")
                    nc.vector.reciprocal(out=rinv, in_=s_sum)
                    ctxn_f = aux.tile([128, TAU], F32, tag="ctxn", name="ctxn_f")
                    nc.scalar.activation(
                        out=ctxn_f, in_=ctx_ps, func=Act.Copy, scale=rinv
                    )
                    ctxT_b = aux.tile([128, CQ, 128], BF16, tag="ctxT", name="ctxT_b")
                    for j in range(CQ):
                        ptb = psum_tr.tile([128, 128], F32, tag="tr", name="ptb")
                        nc.tensor.transpose(
                            ptb, ctxn_f[:, j * 128 : (j + 1) * 128], ident_f
                        )
                        nc.scalar.copy(out=ctxT_b[:, j, :], in_=ptb)
                    pm = psum_out.tile([128, VDIM], F32, tag="mm", name="pm")
                    for j in range(CQ):
                        nc.tensor.matmul(
                            pm,
                            lhsT=ctxT_b[:, j, :],
                            rhs=WvT_b[:, j, :],
                            start=(j == 0),
                            stop=False,
                        )
                    nc.tensor.matmul(
                        pm, lhsT=onespad_b, rhs=bvpad_b, start=False, stop=True
                    )
                    msg_out = aux.tile([128, VDIM], F32, tag="msg", name="msg_out")
                    nc.scalar.copy(out=msg_out, in_=pm)
                    nc.sync.dma_start(out=out[bsl, :], in_=msg_out)

    nc.compile()
    return nc


_NC_CACHE = {}


def _get_nc():
    key = "default"
    if key not in _NC_CACHE:
        _NC_CACHE[key] = build()
    return _NC_CACHE[key]


def make_in_maps(imagined_trajectory, received_messages, Wq, bq, Wk, Wv, bv):
    Wq = np.asarray(Wq, dtype=np.float32)
    bq = np.asarray(bq, dtype=np.float32)
    Wk = np.asarray(Wk, dtype=np.float32)
    Wv = np.asarray(Wv, dtype=np.float32)
    bv = np.asarray(bv, dtype=np.float32)
    # batch-independent weight fusion, done once on host
    Wfused = np.ascontiguousarray(Wq.T @ Wk).astype(NP_BF16)  # [MSG, TAU]
    qkbias = (bq @ Wk).astype(NP_BF16)  # [TAU]
    WvT = np.ascontiguousarray(Wv.T).astype(NP_BF16)  # [TAU, VDIM]

    bl = B_LOCAL
    in_maps = []
    for i in range(N_CORES):
        sl = slice(i * bl, (i + 1) * bl)
        in_maps.append(
            {
                "imagined_trajectory": np.ascontiguousarray(
                    imagined_trajectory[sl], dtype=np.float32
                ),
                "msgsT": np.ascontiguousarray(
                    np.asarray(received_messages[sl], dtype=np.float32).T
                ).astype(NP_BF16),
                "Wfused": Wfused,
                "qkbias": qkbias,
                "WvT": WvT,
                "bv": bv,
            }
        )
    return in_maps


def kernel(
    imagined_trajectory,
    received_messages,
    Wq,
    bq,
    Wk,
    bk,
    Wv,
    bv,
):
    nc = _get_nc()
    in_maps = make_in_maps(
        imagined_trajectory, received_messages, Wq, bq, Wk, Wv, bv
    )
    res = run_bass_kernel_spmd(nc, in_maps, list(range(N_CORES)))
    return np.concatenate([res.results[i]["out"] for i in range(N_CORES)], axis=0)
